# revision 21
# baseline (speedup 1.0000x reference)
"""Trainium2 Bass kernel for a dense transformer block (B=2, S=2048, D=1024,
H=16, d_ff=4096), sharded over 8 NeuronCores.

Sharding: DP(2 groups over batch) x TP(4 cores over heads) for
LN1/QKV/attention/proj, pipelined per 512-token chunk with a per-chunk
ReduceScatter of the proj partials; then token-parallel MLP (each core:
512 tokens, full MLP weights). Host assembles the 8 per-core outputs.

Heavy matmuls run in float32r (TF32-like, full PE rate at N>=512);
LayerNorm transposes (xc^T @ diag(rstd)) run in exact fp32.
"""

from contextlib import ExitStack

import numpy as np

import concourse.bacc as bacc
import concourse.mybir as mybir
import concourse.tile as tile
from concourse.bass_utils import run_bass_kernel_spmd
from concourse.masks import make_identity

f32 = mybir.dt.float32
f32r = mybir.dt.float32r
AF = mybir.ActivationFunctionType
OP = mybir.AluOpType

B = 2
S_FULL = 2048
D = 1024
H = 16
HD = 64
DFF_FULL = 4096
LN_EPS = 1e-5
N_CORES = 8
GROUP_FULL = 4
HPC = 4
DJ = D // 128
CS = 512


def build_nc(S=S_FULL, DFF=DFF_FULL, GROUP=GROUP_FULL, n_cores=N_CORES):
    NCH = S // CS
    SL = S // GROUP
    SLT = SL // 128
    NF = DFF // 128
    KT = S // 128
    CSG = CS // GROUP             # rows per core per chunk after RS
    groups = [list(range(g * GROUP, (g + 1) * GROUP))
              for g in range(n_cores // GROUP)]

    nc = bacc.Bacc("TRN2", target_bir_lowering=False, debug=False,
                   num_devices=n_cores)

    def din(name, shape):
        return nc.dram_tensor(name, shape, f32, kind="ExternalInput").ap()

    x_d = din("x_b", [S, D])
    xo_d = din("x_own", [SL, D])
    g1_d = din("g1_m", [128, DJ])
    b1_d = din("b1_m", [128, DJ])
    g2_d = din("g2_m", [128, DJ])
    b2_d = din("b2_m", [128, DJ])
    wq_d = din("wq_m", [128, DJ, 256])
    wk_d = din("wk_m", [128, DJ, 256])
    wv_d = din("wv_m", [128, DJ, 256])
    bq_d = din("bq_m", [128, 2])
    bk_d = din("bk_m", [128, 2])
    bv_d = din("bv_m", [64, 4])
    wp_d = din("wproj_m", [128, 2, D])
    bp_d = din("b_proj", [D])
    wfc_d = din("wfc_m", [NF, 128, DJ, 128])
    bfc_d = din("bfc_m", [128, NF])
    wo_d = din("wout_m", [DFF, D])
    bo_d = din("b_out", [D])
    out_d = nc.dram_tensor("out_s", [SL, D], f32, kind="ExternalOutput").ap()

    with tile.TileContext(nc) as tc, ExitStack() as st0:
            su = st0.enter_context(tc.tile_pool(name="setup", bufs=1))
            ws = st0.enter_context(tc.tile_pool(name="wstream", bufs=2))
            drp = st0.enter_context(tc.tile_pool(name="dram", bufs=1, space="DRAM"))

            cc_ins = [drp.tile([CS, D], f32, name=f"cc_in{i}")
                      for i in range(NCH)]
            cc_outs = [drp.tile([CSG, D], f32, name=f"cc_out{i}")
                       for i in range(NCH)]

            ident = su.tile([128, 128], f32, name="ident")
            make_identity(nc, ident[:])
            ones_c = su.tile([128, 1], f32, name="ones_c")
            nc.vector.memset(ones_c[:], 1.0)
            negC = su.tile([128, 1], f32, name="negC")
            nc.vector.memset(negC[:], -4.0)
            epsb = su.tile([128, 1], f32, name="epsb")
            nc.vector.memset(epsb[:], LN_EPS)

            g1v = su.tile([128, DJ], f32, name="g1v")
            nc.sync.dma_start(g1v[:], g1_d)
            b1v = su.tile([128, DJ], f32, name="b1v")
            nc.sync.dma_start(b1v[:], b1_d)
            g2v = su.tile([128, DJ], f32, name="g2v")
            nc.sync.dma_start(g2v[:], g2_d)
            b2v = su.tile([128, DJ], f32, name="b2v")
            nc.sync.dma_start(b2v[:], b2_d)
            bq_sb = su.tile([128, 2], f32, name="bq_sb")
            nc.sync.dma_start(bq_sb[:], bq_d)
            bk_sb = su.tile([128, 2], f32, name="bk_sb")
            nc.sync.dma_start(bk_sb[:], bk_d)
            bv_sb = su.tile([64, 4], f32, name="bv_sb")
            nc.sync.dma_start(bv_sb[:], bv_d)
            bfc_sb = su.tile([128, NF], f32, name="bfc_sb")
            nc.sync.dma_start(bfc_sb[:], bfc_d)

            bproj_bc = su.tile([128, D], f32, name="bproj_bc")
            bout_bc = su.tile([128, D], f32, name="bout_bc")
            with tc.tile_pool(name="tmpb", bufs=1) as tb:
                brow = tb.tile([1, 2, D], f32, name="brow")
                nc.sync.dma_start(brow[:, 0, :], bp_d[None, :])
                nc.sync.dma_start(brow[:, 1, :], bo_d[None, :])
                nc.gpsimd.partition_broadcast(bproj_bc[:], brow[:, 0, :])
                nc.gpsimd.partition_broadcast(bout_bc[:], brow[:, 1, :])

            # ------------- fused phases 1-3: per-chunk pipeline -------------
            with ExitStack() as st1:
                ap = st1.enter_context(tc.tile_pool(name="attn_per", bufs=1))
                p1w = st1.enter_context(tc.tile_pool(name="p1w", bufs=1))
                p1x = st1.enter_context(tc.tile_pool(name="p1x", bufs=4))
                p1xc = st1.enter_context(tc.tile_pool(name="p1xc", bufs=4))
                p1ht = st1.enter_context(tc.tile_pool(name="p1ht", bufs=1))
                p1d = st1.enter_context(tc.tile_pool(name="p1d", bufs=8))
                p1s = st1.enter_context(tc.tile_pool(name="p1s", bufs=8))
                pmm = st1.enter_context(tc.tile_pool(name="pmm", bufs=2, space="PSUM"))
                p2sc = st1.enter_context(tc.tile_pool(name="p2sc", bufs=2, space="PSUM"))
                p2y = st1.enter_context(tc.tile_pool(name="p2y", bufs=2, space="PSUM"))
                p2e = st1.enter_context(tc.tile_pool(name="p2e", bufs=2))
                p2t = st1.enter_context(tc.tile_pool(name="p2t", bufs=2))
                p2o = st1.enter_context(tc.tile_pool(name="p2o", bufs=2))

                Qt = ap.tile([128, 2, S], f32r, name="Qt")
                Kt = ap.tile([128, 2, S], f32r, name="Kt")
                yT = ap.tile([128, 2, S], f32r, name="yT")
                Vg = ap.tile([128, KT, HPC, 65], f32r, name="Vg")
                Wp_sb = ap.tile([128, 2, D], f32r, name="Wp_sb")
                nc.sync.dma_start(Wp_sb[:], wp_d.bitcast(f32r))
                Wq_sb = p1w.tile([128, DJ, 256], f32r, name="Wq_sb")
                nc.sync.dma_start(Wq_sb[:], wq_d.bitcast(f32r))
                Wk_sb = p1w.tile([128, DJ, 256], f32r, name="Wk_sb")
                nc.sync.dma_start(Wk_sb[:], wk_d.bitcast(f32r))
                Wv_sb = p1w.tile([128, DJ, 256], f32r, name="Wv_sb")
                nc.sync.dma_start(Wv_sb[:], wv_d.bitcast(f32r))

                for ch in range(NCH):
                    # ---- LN1 stats + diag for the 4 token tiles ----
                    xcs, diags = [], []
                    for tl in range(4):
                        ti = ch * 4 + tl
                        xt = p1x.tile([128, D], f32, name="xt", tag="xt")
                        nc.sync.dma_start(
                            xt[:], x_d[ti * 128:(ti + 1) * 128, :])
                        s1 = p1s.tile([128, 1], f32, name="s1", tag="s1")
                        nc.vector.reduce_sum(
                            s1[:], xt[:], axis=mybir.AxisListType.X)
                        mu = p1s.tile([128, 1], f32, name="mu", tag="mu")
                        nc.vector.tensor_scalar_mul(mu[:], s1[:], 1.0 / D)
                        xc = p1xc.tile([128, D], f32, name="xc", tag="xc")
                        nc.vector.tensor_scalar(
                            xc[:], xt[:], mu[:], None, OP.subtract)
                        nc.vector.tensor_tensor(xt[:], xc[:], xc[:], OP.mult)
                        ss = p1s.tile([128, 1], f32, name="ss", tag="ss")
                        nc.vector.reduce_sum(
                            ss[:], xt[:], axis=mybir.AxisListType.X)
                        sd = p1s.tile([128, 1], f32, name="sd", tag="sd")
                        nc.scalar.activation(
                            sd[:], ss[:], AF.Sqrt, bias=epsb[:], scale=1.0 / D)
                        rstd = p1s.tile([128, 1], f32, name="rstd", tag="rstd")
                        nc.vector.reciprocal(rstd[:], sd[:])
                        dg = p1d.tile([128, 128], f32, name="dg", tag="dg")
                        nc.vector.tensor_scalar_mul(dg[:], ident[:], rstd[:])
                        xcs.append(xc)
                        diags.append(dg)

                    # ---- h^T via diag matmuls ----
                    hT = p1ht.tile([128, DJ, CS], f32r, name="hT", tag="hT")
                    for j in range(DJ):
                        for tl in range(4):
                            ptt = pmm.tile([128, 512], f32, name="ptt",
                                           tag="mm")
                            nc.tensor.matmul(
                                ptt[:, 0:128],
                                xcs[tl][:, j * 128:(j + 1) * 128],
                                diags[tl][:], start=True, stop=True)
                            nc.vector.tensor_scalar(
                                hT[:, j, tl * 128:(tl + 1) * 128],
                                ptt[:, 0:128], g1v[:, j:j + 1],
                                b1v[:, j:j + 1], OP.mult, OP.add)

                    # ---- QKV ----
                    for hp in range(2):
                        psq = pmm.tile([128, 512], f32, name="psq", tag="mm")
                        for j in range(DJ):
                            nc.tensor.matmul(
                                psq[:], Wq_sb[:, j, hp * 128:(hp + 1) * 128],
                                hT[:, j, :], start=(j == 0),
                                stop=(j == DJ - 1))
                        nc.vector.tensor_scalar(
                            Qt[:, hp, ch * CS:(ch + 1) * CS], psq[:],
                            bq_sb[:, hp:hp + 1], None, OP.add)
                        psk = pmm.tile([128, 512], f32, name="psk", tag="mm")
                        for j in range(DJ):
                            nc.tensor.matmul(
                                psk[:], Wk_sb[:, j, hp * 128:(hp + 1) * 128],
                                hT[:, j, :], start=(j == 0),
                                stop=(j == DJ - 1))
                        nc.vector.tensor_scalar(
                            Kt[:, hp, ch * CS:(ch + 1) * CS], psk[:],
                            bk_sb[:, hp:hp + 1], None, OP.add)
                    for tl in range(4):
                        ti = ch * 4 + tl
                        psv = pmm.tile([128, 512], f32, name="psv", tag="mm")
                        for j in range(DJ):
                            nc.tensor.matmul(
                                psv[:, 0:256],
                                hT[:, j, tl * 128:(tl + 1) * 128],
                                Wv_sb[:, j, :], start=(j == 0),
                                stop=(j == DJ - 1))
                        for h in range(HPC):
                            nc.vector.tensor_copy(
                                Vg[:, ti, h, 0:64],
                                psv[:, h * 64:(h + 1) * 64])
                            nc.vector.tensor_copy(
                                Vg[:, ti, h, 64:65], ones_c[:])

                    # ---- attention for qi chunk qc == ch ----
                    qc = ch
                    q0 = qc * CS
                    nkj = (q0 + CS) // 128
                    for hp in range(2):
                        for h2 in range(2):
                            h = hp * 2 + h2
                            psy = p2y.tile([128, CS], f32, name="psy",
                                           tag="psy")
                            first = True
                            for g0 in range(0, nkj, 2):
                                pss = p2sc.tile([128, 1024], f32, name="pss",
                                                tag="pss")
                                for kk in range(2):
                                    kjt = g0 + kk
                                    nc.tensor.matmul(
                                        pss[:, kk * 512:(kk + 1) * 512],
                                        Kt[h2 * 64:(h2 + 1) * 64, hp,
                                           kjt * 128:(kjt + 1) * 128],
                                        Qt[h2 * 64:(h2 + 1) * 64, hp,
                                           q0:q0 + CS],
                                        start=True, stop=True)
                                es = p2e.tile([128, 1024], f32r, name="es",
                                              tag="es")
                                nc.scalar.activation(
                                    es[:], pss[:], AF.Exp, bias=negC[:],
                                    scale=0.125)
                                for kk in range(2):
                                    kjt = g0 + kk
                                    k0 = kjt * 128
                                    if k0 >= q0:
                                        nc.gpsimd.affine_select(
                                            out=es[:, kk * 512:(kk + 1) * 512],
                                            in_=es[:, kk * 512:(kk + 1) * 512],
                                            compare_op=OP.is_ge,
                                            fill=0.0, base=q0 - k0,
                                            pattern=[[1, CS]],
                                            channel_multiplier=-1)
                                    nc.tensor.matmul(
                                        psy[0:65, :], Vg[:, kjt, h, :],
                                        es[:, kk * 512:(kk + 1) * 512],
                                        start=first, stop=(kjt == nkj - 1))
                                    first = False
                            yt65 = p2t.tile([65, CS], f32, name="yt65",
                                            tag="yt65")
                            nc.vector.tensor_copy(yt65[:], psy[0:65, :])
                            iv = p2t.tile([1, CS], f32, name="iv", tag="iv")
                            nc.vector.reciprocal(iv[:], yt65[64:65, :])
                            bcst = p2t.tile([64, CS], f32, name="bcst",
                                            tag="bcst")
                            nc.gpsimd.partition_broadcast(bcst[:], iv[:])
                            stg = p2t.tile([64, CS], f32, name="stg",
                                           tag="stg")
                            nc.vector.tensor_tensor(
                                stg[:], yt65[0:64, :], bcst[:], OP.mult)
                            if h2 == 0:
                                nc.vector.tensor_scalar(
                                    yT[0:64, hp, q0:q0 + CS], stg[:],
                                    bv_sb[:, h:h + 1], None, OP.add)
                            else:
                                st2 = p2t.tile([64, CS], f32r, name="st2",
                                               tag="st2")
                                nc.vector.tensor_scalar(
                                    st2[:], stg[:], bv_sb[:, h:h + 1],
                                    None, OP.add)
                                nc.sync.dma_start(
                                    yT[64:128, hp, q0:q0 + CS], st2[:])

                    # ---- proj + reduce-scatter for this chunk ----
                    for tl in range(4):
                        ti = qc * 4 + tl
                        for n in range(2):
                            psp = pmm.tile([128, 512], f32, name="psp",
                                           tag="mm")
                            for hp in range(2):
                                nc.tensor.matmul(
                                    psp[:],
                                    yT[:, hp, ti * 128:(ti + 1) * 128],
                                    Wp_sb[:, hp, n * 512:(n + 1) * 512],
                                    start=(hp == 0), stop=(hp == 1))
                            po = p2o.tile([128, 512], f32, name="po",
                                          tag="po")
                            nc.vector.tensor_copy(po[:], psp[:])
                            nc.sync.dma_start(
                                cc_ins[qc][tl * 128:(tl + 1) * 128,
                                           n * 512:(n + 1) * 512], po[:])
                    nc.gpsimd.collective_compute(
                        "ReduceScatter", OP.add, replica_groups=groups,
                        ins=[cc_ins[qc][:].opt()],
                        outs=[cc_outs[qc][:].opt()])

            # ------------- phases 4-6: residual + LN2 + MLP -------------
            with ExitStack() as st4:
                p4 = st4.enter_context(tc.tile_pool(name="p4per", bufs=1))
                p4z = st4.enter_context(tc.tile_pool(name="p4z", bufs=2))
                p4xc = st4.enter_context(tc.tile_pool(name="p4xc", bufs=1))
                p4d = st4.enter_context(tc.tile_pool(name="p4d", bufs=4))
                p4s = st4.enter_context(tc.tile_pool(name="p4s", bufs=8))
                p4o = st4.enter_context(tc.tile_pool(name="p4o", bufs=2))

                xP = p4.tile([128, SLT, D], f32, name="xP")
                h2T = p4.tile([128, DJ, SL], f32r, name="h2T")
                m1T = p4.tile([128, NF, SL], f32r, name="m1T")

                xc2s, diag2s = [], []
                for tl in range(SLT):
                    z = p4z.tile([128, D], f32, name="z", tag="z")
                    zq, zr = divmod(tl * 128, CSG)
                    nc.sync.dma_start(z[:], cc_outs[zq][zr:zr + 128, :])
                    xre = p4z.tile([128, D], f32, name="xre", tag="xre")
                    nc.sync.dma_start(
                        xre[:], xo_d[tl * 128:(tl + 1) * 128, :])
                    nc.vector.tensor_tensor(xP[:, tl, :], z[:], xre[:], OP.add)
                    nc.vector.tensor_tensor(
                        xP[:, tl, :], xP[:, tl, :], bproj_bc[:], OP.add)
                    s1b = p4s.tile([128, 1], f32, name="s1b", tag="s1b")
                    nc.vector.reduce_sum(
                        s1b[:], xP[:, tl, :], axis=mybir.AxisListType.X)
                    mu2 = p4s.tile([128, 1], f32, name="mu2", tag="mu2")
                    nc.vector.tensor_scalar_mul(mu2[:], s1b[:], 1.0 / D)
                    xc2 = p4xc.tile([128, D], f32, name="xc2", tag=f"xc2_{tl}")
                    nc.vector.tensor_scalar(
                        xc2[:], xP[:, tl, :], mu2[:], None, OP.subtract)
                    nc.vector.tensor_tensor(xre[:], xc2[:], xc2[:], OP.mult)
                    ss2 = p4s.tile([128, 1], f32, name="ss2", tag="ss2")
                    nc.vector.reduce_sum(
                        ss2[:], xre[:], axis=mybir.AxisListType.X)
                    sd2 = p4s.tile([128, 1], f32, name="sd2", tag="sd2")
                    nc.scalar.activation(
                        sd2[:], ss2[:], AF.Sqrt, bias=epsb[:], scale=1.0 / D)
                    rstd2 = p4s.tile([128, 1], f32, name="rstd2", tag="rstd2")
                    nc.vector.reciprocal(rstd2[:], sd2[:])
                    dg2 = p4d.tile([128, 128], f32, name="dg2", tag="dg2")
                    nc.vector.tensor_scalar_mul(dg2[:], ident[:], rstd2[:])
                    xc2s.append(xc2)
                    diag2s.append(dg2)

                with tc.tile_pool(name="p45ps", bufs=2, space="PSUM") as p45ps:
                    for j in range(DJ):
                        for tl in range(SLT):
                            pt2 = p45ps.tile([128, 128], f32, name="pt2",
                                             tag="pt2")
                            nc.tensor.matmul(
                                pt2[:], xc2s[tl][:, j * 128:(j + 1) * 128],
                                diag2s[tl][:], start=True, stop=True)
                            nc.vector.tensor_scalar(
                                h2T[:, j, tl * 128:(tl + 1) * 128], pt2[:],
                                g2v[:, j:j + 1], b2v[:, j:j + 1],
                                OP.mult, OP.add)

                    for f in range(NF):
                        wf = ws.tile([128, DJ, 128], f32r, name="wf", tag="wf")
                        nc.sync.dma_start(wf[:], wfc_d[f].bitcast(f32r))
                        psf = p45ps.tile([128, SL], f32, name="psf",
                                         tag="psf")
                        for j in range(DJ):
                            nc.tensor.matmul(
                                psf[:], wf[:, j, :], h2T[:, j, :],
                                start=(j == 0), stop=(j == DJ - 1))
                        nc.vector.tensor_scalar(
                            m1T[:, f, :], psf[:], bfc_sb[:, f:f + 1], 0.0,
                            OP.add, OP.max)

                with tc.tile_pool(name="p6ps", bufs=1, space="PSUM") as p6ps:
                    pso = [[p6ps.tile([128, 512], f32, name=f"pso_{tl}_{n}")
                            for n in range(2)] for tl in range(SLT)]
                    for f in range(NF):
                        wo = ws.tile([128, D], f32r, name="wo", tag="wo")
                        nc.sync.dma_start(
                            wo[:], wo_d[f * 128:(f + 1) * 128, :].bitcast(f32r))
                        for tl in range(SLT):
                            for n in range(2):
                                nc.tensor.matmul(
                                    pso[tl][n][:],
                                    m1T[:, f, tl * 128:(tl + 1) * 128],
                                    wo[:, n * 512:(n + 1) * 512],
                                    start=(f == 0), stop=(f == NF - 1))
                    for tl in range(SLT):
                        for n in range(2):
                            ot = p4o.tile([128, 512], f32, name="ot", tag="ot")
                            nc.vector.tensor_tensor(
                                ot[:], pso[tl][n][:],
                                xP[:, tl, n * 512:(n + 1) * 512], OP.add)
                            nc.vector.tensor_tensor(
                                ot[:], ot[:],
                                bout_bc[:, n * 512:(n + 1) * 512], OP.add)
                            nc.sync.dma_start(
                                out_d[tl * 128:(tl + 1) * 128,
                                      n * 512:(n + 1) * 512], ot[:])
    nc.compile()
    return nc


def own_token_idx(t, S=S_FULL, GROUP=GROUP_FULL):
    CSG = CS // GROUP
    return np.concatenate([
        np.arange(qc * CS + t * CSG, qc * CS + (t + 1) * CSG)
        for qc in range(S // CS)])


def marshal_inputs(x, ln1_g, ln1_b, ln2_g, ln2_b, W_qkv, b_qkv, W_proj,
                   b_proj, W_fc, b_fc, W_out, b_out,
                   S=S_FULL, DFF=DFF_FULL, GROUP=GROUP_FULL,
                   n_cores=N_CORES):
    NF = DFF // 128
    f32c = np.ascontiguousarray

    def ln_m(v):
        return f32c(v.reshape(DJ, 128).T)

    base = {
        "g1_m": ln_m(ln1_g), "b1_m": ln_m(ln1_b),
        "g2_m": ln_m(ln2_g), "b2_m": ln_m(ln2_b),
        "bfc_m": f32c(b_fc.reshape(NF, 128).T),
        "wfc_m": f32c(W_fc.reshape(DJ, 128, NF, 128).transpose(2, 1, 0, 3)),
        "wout_m": f32c(W_out),
        "b_proj": f32c(b_proj), "b_out": f32c(b_out),
    }
    in_maps = []
    for c in range(n_cores):
        g, t = c // GROUP, c % GROUP
        cs, ce = t * 256, (t + 1) * 256
        wq = W_qkv[:, cs:ce]
        wk = W_qkv[:, D + cs:D + ce]
        wv = W_qkv[:, 2 * D + cs:2 * D + ce]
        bq = b_qkv[cs:ce]
        bk = b_qkv[D + cs:D + ce]
        bv = b_qkv[2 * D + cs:2 * D + ce]
        wp = W_proj[cs:ce, :]
        m = dict(base)
        m["x_b"] = f32c(x[g])
        m["x_own"] = f32c(x[g][own_token_idx(t, S, GROUP)])
        m["wq_m"] = f32c(wq.reshape(DJ, 128, 256).transpose(1, 0, 2))
        m["wk_m"] = f32c(wk.reshape(DJ, 128, 256).transpose(1, 0, 2))
        m["wv_m"] = f32c(wv.reshape(DJ, 128, 256).transpose(1, 0, 2))
        m["bq_m"] = f32c(bq.reshape(2, 128).T)
        m["bk_m"] = f32c(bk.reshape(2, 128).T)
        m["bv_m"] = f32c(bv.reshape(4, 64).T)
        m["wproj_m"] = f32c(
            wp.reshape(2, 2, 64, D).transpose(1, 2, 0, 3).reshape(128, 2, D))
        in_maps.append(m)
    return in_maps


_NC_CACHE = {}


def _get_nc():
    if "nc" not in _NC_CACHE:
        _NC_CACHE["nc"] = build_nc()
    return _NC_CACHE["nc"]


def kernel(**inputs):
    inputs = {k: np.asarray(v, dtype=np.float32) for k, v in inputs.items()}
    nc = _get_nc()
    in_maps = marshal_inputs(**inputs)
    r = run_bass_kernel_spmd(nc, in_maps, core_ids=list(range(N_CORES)))
    out = np.empty((B, S_FULL, D), np.float32)
    for c in range(N_CORES):
        g, t = c // GROUP_FULL, c % GROUP_FULL
        out[g, own_token_idx(t), :] = r.results[c]["out_s"]
    return out


# revision 22
# speedup vs baseline: 1.0701x; 1.0701x over previous
"""Trainium2 Bass kernel for a dense transformer block (B=2, S=2048, D=1024,
H=16, d_ff=4096), sharded over 8 NeuronCores.

Sharding: DP(2 groups over batch) x TP(4 cores over heads) for
LN1/QKV/attention/proj, pipelined per 512-token chunk with a per-chunk
ReduceScatter of the proj partials; then token-parallel MLP (each core:
512 tokens, full MLP weights). Host assembles the 8 per-core outputs.

Heavy matmuls run in float32r (TF32-like, full PE rate at N>=512);
LayerNorm transposes (xc^T @ diag(rstd)) run in exact fp32.
"""

from contextlib import ExitStack

import numpy as np

import concourse.bacc as bacc
import concourse.mybir as mybir
import concourse.tile as tile
from concourse.bass_utils import run_bass_kernel_spmd
from concourse.masks import make_identity

f32 = mybir.dt.float32
f32r = mybir.dt.float32r
AF = mybir.ActivationFunctionType
OP = mybir.AluOpType

B = 2
S_FULL = 2048
D = 1024
H = 16
HD = 64
DFF_FULL = 4096
LN_EPS = 1e-5
N_CORES = 8
GROUP_FULL = 4
HPC = 4
DJ = D // 128
CS = 512


def build_nc(S=S_FULL, DFF=DFF_FULL, GROUP=GROUP_FULL, n_cores=N_CORES):
    NCH = S // CS
    SL = S // GROUP
    SLT = SL // 128
    NF = DFF // 128
    KT = S // 128
    CSG = CS // GROUP             # rows per core per chunk after RS
    groups = [list(range(g * GROUP, (g + 1) * GROUP))
              for g in range(n_cores // GROUP)]

    nc = bacc.Bacc("TRN2", target_bir_lowering=False, debug=False,
                   num_devices=n_cores)

    def din(name, shape):
        return nc.dram_tensor(name, shape, f32, kind="ExternalInput").ap()

    x_d = din("x_b", [S, D])
    xo_d = din("x_own", [SL, D])
    g1_d = din("g1_m", [128, DJ])
    b1_d = din("b1_m", [128, DJ])
    g2_d = din("g2_m", [128, DJ])
    b2_d = din("b2_m", [128, DJ])
    wq_d = din("wq_m", [128, DJ, 256])
    wk_d = din("wk_m", [128, DJ, 256])
    wv_d = din("wv_m", [128, DJ, 256])
    bq_d = din("bq_m", [128, 2])
    bk_d = din("bk_m", [128, 2])
    bv_d = din("bv_m", [64, 4])
    wp_d = din("wproj_m", [128, 2, D])
    bp_d = din("b_proj", [D])
    wfc_d = din("wfc_m", [NF, 128, DJ, 128])
    bfc_d = din("bfc_m", [128, NF])
    wo_d = din("wout_m", [DFF, D])
    bo_d = din("b_out", [D])
    out_d = nc.dram_tensor("out_s", [SL, D], f32, kind="ExternalOutput").ap()

    with tile.TileContext(nc) as tc, ExitStack() as st0:
            su = st0.enter_context(tc.tile_pool(name="setup", bufs=1))
            ws = st0.enter_context(tc.tile_pool(name="wstream", bufs=2))
            drp = st0.enter_context(tc.tile_pool(name="dram", bufs=1, space="DRAM"))

            cc_ins = [drp.tile([CS, D], f32, name=f"cc_in{i}")
                      for i in range(NCH)]
            cc_outs = [drp.tile([CSG, D], f32, name=f"cc_out{i}")
                       for i in range(NCH)]

            ident = su.tile([128, 128], f32, name="ident")
            make_identity(nc, ident[:])
            ones_c = su.tile([128, 1], f32, name="ones_c")
            nc.vector.memset(ones_c[:], 1.0)
            negC = su.tile([128, 1], f32, name="negC")
            nc.vector.memset(negC[:], -4.0)
            epsb = su.tile([128, 1], f32, name="epsb")
            nc.vector.memset(epsb[:], LN_EPS)

            g1v = su.tile([128, DJ], f32, name="g1v")
            nc.sync.dma_start(g1v[:], g1_d)
            b1v = su.tile([128, DJ], f32, name="b1v")
            nc.sync.dma_start(b1v[:], b1_d)
            g2v = su.tile([128, DJ], f32, name="g2v")
            nc.sync.dma_start(g2v[:], g2_d)
            b2v = su.tile([128, DJ], f32, name="b2v")
            nc.sync.dma_start(b2v[:], b2_d)
            bq_sb = su.tile([128, 2], f32, name="bq_sb")
            nc.sync.dma_start(bq_sb[:], bq_d)
            bk_sb = su.tile([128, 2], f32, name="bk_sb")
            nc.sync.dma_start(bk_sb[:], bk_d)
            bv_sb = su.tile([64, 4], f32, name="bv_sb")
            nc.sync.dma_start(bv_sb[:], bv_d)
            bfc_sb = su.tile([128, NF], f32, name="bfc_sb")
            nc.sync.dma_start(bfc_sb[:], bfc_d)

            bproj_bc = su.tile([128, D], f32, name="bproj_bc")
            bout_bc = su.tile([128, D], f32, name="bout_bc")
            with tc.tile_pool(name="tmpb", bufs=1) as tb:
                brow = tb.tile([1, 2, D], f32, name="brow")
                nc.sync.dma_start(brow[:, 0, :], bp_d[None, :])
                nc.sync.dma_start(brow[:, 1, :], bo_d[None, :])
                nc.gpsimd.partition_broadcast(bproj_bc[:], brow[:, 0, :])
                nc.gpsimd.partition_broadcast(bout_bc[:], brow[:, 1, :])

            # ------------- fused phases 1-3: per-chunk pipeline -------------
            with ExitStack() as st1:
                ap = st1.enter_context(tc.tile_pool(name="attn_per", bufs=1))
                p1w = st1.enter_context(tc.tile_pool(name="p1w", bufs=1))
                p1x = st1.enter_context(tc.tile_pool(name="p1x", bufs=4))
                p1xc = st1.enter_context(tc.tile_pool(name="p1xc", bufs=4))
                p1ht = st1.enter_context(tc.tile_pool(name="p1ht", bufs=1))
                p1d = st1.enter_context(tc.tile_pool(name="p1d", bufs=8))
                p1s = st1.enter_context(tc.tile_pool(name="p1s", bufs=8))
                pmm = st1.enter_context(tc.tile_pool(name="pmm", bufs=2, space="PSUM"))
                p2sc = st1.enter_context(tc.tile_pool(name="p2sc", bufs=2, space="PSUM"))
                p2y = st1.enter_context(tc.tile_pool(name="p2y", bufs=1, space="PSUM"))
                p2e = st1.enter_context(tc.tile_pool(name="p2e", bufs=2))
                p2t = st1.enter_context(tc.tile_pool(name="p2t", bufs=2))
                p2o = st1.enter_context(tc.tile_pool(name="p2o", bufs=2))

                Qt = ap.tile([128, 2, S], f32r, name="Qt")
                Kt = ap.tile([128, 2, S], f32r, name="Kt")
                yT = ap.tile([128, 2, S], f32r, name="yT")
                Vg = ap.tile([128, KT, HPC, 65], f32r, name="Vg")
                Wp_sb = ap.tile([128, 2, D], f32r, name="Wp_sb")
                nc.sync.dma_start(Wp_sb[:], wp_d.bitcast(f32r))
                Wq_sb = p1w.tile([128, DJ, 256], f32r, name="Wq_sb")
                nc.sync.dma_start(Wq_sb[:], wq_d.bitcast(f32r))
                Wk_sb = p1w.tile([128, DJ, 256], f32r, name="Wk_sb")
                nc.sync.dma_start(Wk_sb[:], wk_d.bitcast(f32r))
                Wv_sb = p1w.tile([128, DJ, 256], f32r, name="Wv_sb")
                nc.sync.dma_start(Wv_sb[:], wv_d.bitcast(f32r))

                for ch in range(NCH):
                    # ---- LN1 stats + diag for the 4 token tiles ----
                    xcs, diags = [], []
                    for tl in range(4):
                        ti = ch * 4 + tl
                        xt = p1x.tile([128, D], f32, name="xt", tag="xt")
                        nc.sync.dma_start(
                            xt[:], x_d[ti * 128:(ti + 1) * 128, :])
                        s1 = p1s.tile([128, 1], f32, name="s1", tag="s1")
                        nc.vector.reduce_sum(
                            s1[:], xt[:], axis=mybir.AxisListType.X)
                        mu = p1s.tile([128, 1], f32, name="mu", tag="mu")
                        nc.vector.tensor_scalar_mul(mu[:], s1[:], 1.0 / D)
                        xc = p1xc.tile([128, D], f32, name="xc", tag="xc")
                        nc.vector.tensor_scalar(
                            xc[:], xt[:], mu[:], None, OP.subtract)
                        nc.vector.tensor_tensor(xt[:], xc[:], xc[:], OP.mult)
                        ss = p1s.tile([128, 1], f32, name="ss", tag="ss")
                        nc.vector.reduce_sum(
                            ss[:], xt[:], axis=mybir.AxisListType.X)
                        sd = p1s.tile([128, 1], f32, name="sd", tag="sd")
                        nc.scalar.activation(
                            sd[:], ss[:], AF.Sqrt, bias=epsb[:], scale=1.0 / D)
                        rstd = p1s.tile([128, 1], f32, name="rstd", tag="rstd")
                        nc.vector.reciprocal(rstd[:], sd[:])
                        dg = p1d.tile([128, 128], f32, name="dg", tag="dg")
                        nc.vector.tensor_scalar_mul(dg[:], ident[:], rstd[:])
                        xcs.append(xc)
                        diags.append(dg)

                    # ---- h^T via diag matmuls ----
                    hT = p1ht.tile([128, DJ, CS], f32r, name="hT", tag="hT")
                    for j in range(DJ):
                        ptt = pmm.tile([128, 512], f32, name="ptt", tag="mm")
                        for tl in range(4):
                            nc.tensor.matmul(
                                ptt[:, tl * 128:(tl + 1) * 128],
                                xcs[tl][:, j * 128:(j + 1) * 128],
                                diags[tl][:], start=True, stop=True)
                        nc.vector.tensor_scalar(
                            hT[:, j, :], ptt[:], g1v[:, j:j + 1],
                            b1v[:, j:j + 1], OP.mult, OP.add)

                    # ---- QKV ----
                    for hp in range(2):
                        psq = pmm.tile([128, 512], f32, name="psq", tag="mm")
                        for j in range(DJ):
                            nc.tensor.matmul(
                                psq[:], Wq_sb[:, j, hp * 128:(hp + 1) * 128],
                                hT[:, j, :], start=(j == 0),
                                stop=(j == DJ - 1))
                        nc.vector.tensor_scalar(
                            Qt[:, hp, ch * CS:(ch + 1) * CS], psq[:],
                            bq_sb[:, hp:hp + 1], None, OP.add)
                        psk = pmm.tile([128, 512], f32, name="psk", tag="mm")
                        for j in range(DJ):
                            nc.tensor.matmul(
                                psk[:], Wk_sb[:, j, hp * 128:(hp + 1) * 128],
                                hT[:, j, :], start=(j == 0),
                                stop=(j == DJ - 1))
                        nc.vector.tensor_scalar(
                            Kt[:, hp, ch * CS:(ch + 1) * CS], psk[:],
                            bk_sb[:, hp:hp + 1], None, OP.add)
                    for tl in range(4):
                        ti = ch * 4 + tl
                        psv = pmm.tile([128, 512], f32, name="psv", tag="mm")
                        for j in range(DJ):
                            nc.tensor.matmul(
                                psv[:, 0:256],
                                hT[:, j, tl * 128:(tl + 1) * 128],
                                Wv_sb[:, j, :], start=(j == 0),
                                stop=(j == DJ - 1))
                        for h in range(HPC):
                            nc.vector.tensor_copy(
                                Vg[:, ti, h, 0:64],
                                psv[:, h * 64:(h + 1) * 64])
                            nc.vector.tensor_copy(
                                Vg[:, ti, h, 64:65], ones_c[:])

                    # ---- attention for qi chunk qc == ch ----
                    qc = ch
                    q0 = qc * CS
                    nkj = (q0 + CS) // 128
                    for hp in range(2):
                        for h2 in range(2):
                            h = hp * 2 + h2
                            psy = p2y.tile([128, CS], f32, name="psy",
                                           tag="psy")
                            first = True
                            for g0 in range(0, nkj, 2):
                                pss = p2sc.tile([128, 1024], f32, name="pss",
                                                tag="pss")
                                for kk in range(2):
                                    kjt = g0 + kk
                                    nc.tensor.matmul(
                                        pss[:, kk * 512:(kk + 1) * 512],
                                        Kt[h2 * 64:(h2 + 1) * 64, hp,
                                           kjt * 128:(kjt + 1) * 128],
                                        Qt[h2 * 64:(h2 + 1) * 64, hp,
                                           q0:q0 + CS],
                                        start=True, stop=True)
                                es = p2e.tile([128, 1024], f32r, name="es",
                                              tag="es")
                                nc.scalar.activation(
                                    es[:], pss[:], AF.Exp, bias=negC[:],
                                    scale=0.125)
                                for kk in range(2):
                                    kjt = g0 + kk
                                    k0 = kjt * 128
                                    if k0 >= q0:
                                        nc.gpsimd.affine_select(
                                            out=es[:, kk * 512:(kk + 1) * 512],
                                            in_=es[:, kk * 512:(kk + 1) * 512],
                                            compare_op=OP.is_ge,
                                            fill=0.0, base=q0 - k0,
                                            pattern=[[1, CS]],
                                            channel_multiplier=-1)
                                    nc.tensor.matmul(
                                        psy[0:65, :], Vg[:, kjt, h, :],
                                        es[:, kk * 512:(kk + 1) * 512],
                                        start=first, stop=(kjt == nkj - 1))
                                    first = False
                            yt65 = p2t.tile([65, CS], f32, name="yt65",
                                            tag="yt65")
                            nc.vector.tensor_copy(yt65[:], psy[0:65, :])
                            iv = p2t.tile([1, CS], f32, name="iv", tag="iv")
                            nc.vector.reciprocal(iv[:], yt65[64:65, :])
                            bcst = p2t.tile([64, CS], f32, name="bcst",
                                            tag="bcst")
                            nc.gpsimd.partition_broadcast(bcst[:], iv[:])
                            stg = p2t.tile([64, CS], f32, name="stg",
                                           tag="stg")
                            nc.vector.tensor_tensor(
                                stg[:], yt65[0:64, :], bcst[:], OP.mult)
                            if h2 == 0:
                                nc.vector.tensor_scalar(
                                    yT[0:64, hp, q0:q0 + CS], stg[:],
                                    bv_sb[:, h:h + 1], None, OP.add)
                            else:
                                st2 = p2t.tile([64, CS], f32r, name="st2",
                                               tag="st2")
                                nc.vector.tensor_scalar(
                                    st2[:], stg[:], bv_sb[:, h:h + 1],
                                    None, OP.add)
                                nc.sync.dma_start(
                                    yT[64:128, hp, q0:q0 + CS], st2[:])

                    # ---- proj + reduce-scatter for this chunk ----
                    for tl in range(4):
                        ti = qc * 4 + tl
                        for n in range(2):
                            psp = pmm.tile([128, 512], f32, name="psp",
                                           tag="mm")
                            for hp in range(2):
                                nc.tensor.matmul(
                                    psp[:],
                                    yT[:, hp, ti * 128:(ti + 1) * 128],
                                    Wp_sb[:, hp, n * 512:(n + 1) * 512],
                                    start=(hp == 0), stop=(hp == 1))
                            po = p2o.tile([128, 512], f32, name="po",
                                          tag="po")
                            nc.vector.tensor_copy(po[:], psp[:])
                            nc.sync.dma_start(
                                cc_ins[qc][tl * 128:(tl + 1) * 128,
                                           n * 512:(n + 1) * 512], po[:])
                    nc.gpsimd.collective_compute(
                        "ReduceScatter", OP.add, replica_groups=groups,
                        ins=[cc_ins[qc][:].opt()],
                        outs=[cc_outs[qc][:].opt()])

            # ------------- phases 4-6: residual + LN2 + MLP -------------
            with ExitStack() as st4:
                p4 = st4.enter_context(tc.tile_pool(name="p4per", bufs=1))
                p4z = st4.enter_context(tc.tile_pool(name="p4z", bufs=2))
                p4xc = st4.enter_context(tc.tile_pool(name="p4xc", bufs=1))
                p4d = st4.enter_context(tc.tile_pool(name="p4d", bufs=4))
                p4s = st4.enter_context(tc.tile_pool(name="p4s", bufs=8))
                p4o = st4.enter_context(tc.tile_pool(name="p4o", bufs=2))

                xP = p4.tile([128, SLT, D], f32, name="xP")
                h2T = p4.tile([128, DJ, SL], f32r, name="h2T")
                m1T = p4.tile([128, NF, SL], f32r, name="m1T")

                xc2s, diag2s = [], []
                for tl in range(SLT):
                    z = p4z.tile([128, D], f32, name="z", tag="z")
                    zq, zr = divmod(tl * 128, CSG)
                    nc.sync.dma_start(z[:], cc_outs[zq][zr:zr + 128, :])
                    xre = p4z.tile([128, D], f32, name="xre", tag="xre")
                    nc.sync.dma_start(
                        xre[:], xo_d[tl * 128:(tl + 1) * 128, :])
                    nc.vector.tensor_tensor(xP[:, tl, :], z[:], xre[:], OP.add)
                    nc.vector.tensor_tensor(
                        xP[:, tl, :], xP[:, tl, :], bproj_bc[:], OP.add)
                    s1b = p4s.tile([128, 1], f32, name="s1b", tag="s1b")
                    nc.vector.reduce_sum(
                        s1b[:], xP[:, tl, :], axis=mybir.AxisListType.X)
                    mu2 = p4s.tile([128, 1], f32, name="mu2", tag="mu2")
                    nc.vector.tensor_scalar_mul(mu2[:], s1b[:], 1.0 / D)
                    xc2 = p4xc.tile([128, D], f32, name="xc2", tag=f"xc2_{tl}")
                    nc.vector.tensor_scalar(
                        xc2[:], xP[:, tl, :], mu2[:], None, OP.subtract)
                    nc.vector.tensor_tensor(xre[:], xc2[:], xc2[:], OP.mult)
                    ss2 = p4s.tile([128, 1], f32, name="ss2", tag="ss2")
                    nc.vector.reduce_sum(
                        ss2[:], xre[:], axis=mybir.AxisListType.X)
                    sd2 = p4s.tile([128, 1], f32, name="sd2", tag="sd2")
                    nc.scalar.activation(
                        sd2[:], ss2[:], AF.Sqrt, bias=epsb[:], scale=1.0 / D)
                    rstd2 = p4s.tile([128, 1], f32, name="rstd2", tag="rstd2")
                    nc.vector.reciprocal(rstd2[:], sd2[:])
                    dg2 = p4d.tile([128, 128], f32, name="dg2", tag="dg2")
                    nc.vector.tensor_scalar_mul(dg2[:], ident[:], rstd2[:])
                    xc2s.append(xc2)
                    diag2s.append(dg2)

                with tc.tile_pool(name="p45ps", bufs=2, space="PSUM") as p45ps:
                    for j in range(DJ):
                        pt2 = p45ps.tile([128, SL], f32, name="pt2",
                                         tag="pt2")
                        for tl in range(SLT):
                            nc.tensor.matmul(
                                pt2[:, tl * 128:(tl + 1) * 128],
                                xc2s[tl][:, j * 128:(j + 1) * 128],
                                diag2s[tl][:], start=True, stop=True)
                        nc.vector.tensor_scalar(
                            h2T[:, j, :], pt2[:],
                            g2v[:, j:j + 1], b2v[:, j:j + 1],
                            OP.mult, OP.add)

                    for f in range(NF):
                        wf = ws.tile([128, DJ, 128], f32r, name="wf", tag="wf")
                        nc.sync.dma_start(wf[:], wfc_d[f].bitcast(f32r))
                        psf = p45ps.tile([128, SL], f32, name="psf",
                                         tag="psf")
                        for j in range(DJ):
                            nc.tensor.matmul(
                                psf[:], wf[:, j, :], h2T[:, j, :],
                                start=(j == 0), stop=(j == DJ - 1))
                        nc.vector.tensor_scalar(
                            m1T[:, f, :], psf[:], bfc_sb[:, f:f + 1], 0.0,
                            OP.add, OP.max)

                with tc.tile_pool(name="p6ps", bufs=1, space="PSUM") as p6ps:
                    pso = [[p6ps.tile([128, 512], f32, name=f"pso_{tl}_{n}")
                            for n in range(2)] for tl in range(SLT)]
                    for f in range(NF):
                        wo = ws.tile([128, D], f32r, name="wo", tag="wo")
                        nc.sync.dma_start(
                            wo[:], wo_d[f * 128:(f + 1) * 128, :].bitcast(f32r))
                        for tl in range(SLT):
                            for n in range(2):
                                nc.tensor.matmul(
                                    pso[tl][n][:],
                                    m1T[:, f, tl * 128:(tl + 1) * 128],
                                    wo[:, n * 512:(n + 1) * 512],
                                    start=(f == 0), stop=(f == NF - 1))
                    for tl in range(SLT):
                        for n in range(2):
                            ot = p4o.tile([128, 512], f32, name="ot", tag="ot")
                            nc.vector.tensor_tensor(
                                ot[:], pso[tl][n][:],
                                xP[:, tl, n * 512:(n + 1) * 512], OP.add)
                            nc.vector.tensor_tensor(
                                ot[:], ot[:],
                                bout_bc[:, n * 512:(n + 1) * 512], OP.add)
                            nc.sync.dma_start(
                                out_d[tl * 128:(tl + 1) * 128,
                                      n * 512:(n + 1) * 512], ot[:])
    nc.compile()
    return nc


def own_token_idx(t, S=S_FULL, GROUP=GROUP_FULL):
    CSG = CS // GROUP
    return np.concatenate([
        np.arange(qc * CS + t * CSG, qc * CS + (t + 1) * CSG)
        for qc in range(S // CS)])


def marshal_inputs(x, ln1_g, ln1_b, ln2_g, ln2_b, W_qkv, b_qkv, W_proj,
                   b_proj, W_fc, b_fc, W_out, b_out,
                   S=S_FULL, DFF=DFF_FULL, GROUP=GROUP_FULL,
                   n_cores=N_CORES):
    NF = DFF // 128
    f32c = np.ascontiguousarray

    def ln_m(v):
        return f32c(v.reshape(DJ, 128).T)

    base = {
        "g1_m": ln_m(ln1_g), "b1_m": ln_m(ln1_b),
        "g2_m": ln_m(ln2_g), "b2_m": ln_m(ln2_b),
        "bfc_m": f32c(b_fc.reshape(NF, 128).T),
        "wfc_m": f32c(W_fc.reshape(DJ, 128, NF, 128).transpose(2, 1, 0, 3)),
        "wout_m": f32c(W_out),
        "b_proj": f32c(b_proj), "b_out": f32c(b_out),
    }
    in_maps = []
    for c in range(n_cores):
        g, t = c // GROUP, c % GROUP
        cs, ce = t * 256, (t + 1) * 256
        wq = W_qkv[:, cs:ce]
        wk = W_qkv[:, D + cs:D + ce]
        wv = W_qkv[:, 2 * D + cs:2 * D + ce]
        bq = b_qkv[cs:ce]
        bk = b_qkv[D + cs:D + ce]
        bv = b_qkv[2 * D + cs:2 * D + ce]
        wp = W_proj[cs:ce, :]
        m = dict(base)
        m["x_b"] = f32c(x[g])
        m["x_own"] = f32c(x[g][own_token_idx(t, S, GROUP)])
        m["wq_m"] = f32c(wq.reshape(DJ, 128, 256).transpose(1, 0, 2))
        m["wk_m"] = f32c(wk.reshape(DJ, 128, 256).transpose(1, 0, 2))
        m["wv_m"] = f32c(wv.reshape(DJ, 128, 256).transpose(1, 0, 2))
        m["bq_m"] = f32c(bq.reshape(2, 128).T)
        m["bk_m"] = f32c(bk.reshape(2, 128).T)
        m["bv_m"] = f32c(bv.reshape(4, 64).T)
        m["wproj_m"] = f32c(
            wp.reshape(2, 2, 64, D).transpose(1, 2, 0, 3).reshape(128, 2, D))
        in_maps.append(m)
    return in_maps


_NC_CACHE = {}


def _get_nc():
    if "nc" not in _NC_CACHE:
        _NC_CACHE["nc"] = build_nc()
    return _NC_CACHE["nc"]


def kernel(**inputs):
    inputs = {k: np.asarray(v, dtype=np.float32) for k, v in inputs.items()}
    nc = _get_nc()
    in_maps = marshal_inputs(**inputs)
    r = run_bass_kernel_spmd(nc, in_maps, core_ids=list(range(N_CORES)))
    out = np.empty((B, S_FULL, D), np.float32)
    for c in range(N_CORES):
        g, t = c // GROUP_FULL, c % GROUP_FULL
        out[g, own_token_idx(t), :] = r.results[c]["out_s"]
    return out
